# revision 9
# baseline (speedup 1.0000x reference)
"""Trainium2 Bass kernel for nn_LowRankKVCache (prefill path).

The reference computes, for S == MAX_SEQ and right = eye(RANK, D):
    k_full[..., :RANK] = key_states[..., :RANK];  k_full[..., RANK:] = 0
    v_full[..., :RANK] = value_states[..., :RANK]; v_full[..., RANK:] = 0
i.e. a pure memory operation. The 32 (batch, head) pairs are sharded
4-per-core across 8 cores.

Device layout per core: [2, PP*S*RANK] f32 per tensor, where row 0 holds
the data half (in[..., :RANK] flattened) and row 1 is the zero half. This
makes the zero region one contiguous 4 MiB run instead of 256B runs at
512B pitch — strided 256B writes measure ~220 GB/s/core on HW while
contiguous writes saturate the per-core HBM write roofline (~360 GB/s
sustained, ~23 us for the 8 MiB).

Primary path (fast): the device inputs are DONATED, and XLA aliases each
output buffer onto its same-shaped input buffer — so the data rows are
already in place and the NEFF only writes zeros to the row-1 halves
(8 MiB/core of pure contiguous HBM writes): K's on the sync HWDGE ring,
V's on the scalar ring, concurrently. The result is fully validated on
the host (data rows equal inputs, zero rows zero).

Fallback path: if aliasing does not apply in some environment or
validation fails, rerun with a self-contained kernel that copies the
data rows DRAM->DRAM and zero-fills the row-1 halves explicitly (output
buffers are NOT pre-zeroed on the axon/PJRT path).
"""
import numpy as np

import concourse.bass as bass
import concourse.mybir as mybir
from concourse.bass_utils import run_bass_kernel_spmd

_B, _H, _S, _D = 4, 8, 4096, 128
_RANK = 64
_N_CORES = 8
_PP = (_B * _H) // _N_CORES     # (b,h) pairs per core
_NFLAT = _PP * _S * _RANK       # 1048576 elems = 4 MiB per half
_HALF = _NFLAT // 2
F32 = mybir.dt.float32


def _decl(nc):
    k_in = nc.declare_dram_parameter("k_in", [2, _NFLAT], F32, isOutput=False)
    v_in = nc.declare_dram_parameter("v_in", [2, _NFLAT], F32, isOutput=False)
    k_out = nc.declare_dram_parameter("k_out", [2, _NFLAT], F32, isOutput=True)
    v_out = nc.declare_dram_parameter("v_out", [2, _NFLAT], F32, isOutput=True)
    return k_in, v_in, k_out, v_out


def _build_zero_dyn() -> bass.Bass:
    """Zero only the row-1 halves; data rows arrive via buffer aliasing.

    K's 4 MiB zero run goes on the sync HWDGE ring, V's on the scalar
    ring. The whole zero pass repeats `niters` times, with the count read at runtime
    from the `niters` input — so ONE loaded executable serves both the
    real call (niters=1) and the timing sweep. (Identical NEFFs can differ
    by several ms in per-load dispatch constant; a dynamic count makes the
    constant cancel exactly in the timing delta.)

    Loop body iterations are pipelined: body DMAs inc a never-awaited
    semaphore; only the final iteration's DMAs inc the done-semaphore.
    HWDGE descriptors drain FIFO per ring slot, so final-DMA completion
    implies all earlier writes landed."""
    nc = bass.Bass()
    k_in, v_in, k_out, v_out = _decl(nc)
    n_t = nc.declare_dram_parameter("niters", [1, 1], mybir.dt.uint32, isOutput=False)
    with (
        nc.sbuf_tensor([128, 2048], F32) as zt,
        nc.sbuf_tensor([1, 1], mybir.dt.uint32) as nt_sb,
        nc.Block() as block,
        nc.semaphore("sem_z") as sem_z,
        nc.semaphore("sem_nt") as sem_nt,
        nc.semaphore("sem_k") as sem_k,
        nc.semaphore("sem_kd") as sem_kd,
        nc.semaphore("sem_v") as sem_v,
        nc.semaphore("sem_vd") as sem_vd,
    ):
        rep = _NFLAT // (128 * 2048)  # broadcast reps covering one 4 MiB half
        zsrc = zt[:].rearrange("p (o c) -> p o c", o=1).broadcast_to([128, rep, 2048])

        @block.vector
        def _(vec):
            vec.memset(zt[:], 0.0).then_inc(sem_z, 1)

        def eng_loop(eng, name, dst, sem_b, sem_d):
            """Issue pass 1 before the niters value is even loaded (every
            invocation does >=1 pass), hiding the niters DMA+load latency
            behind the first 4 MiB write. Completion is proven by a tiny
            fence DMA on the same ring (64B of zeros over an already-zeroed
            prefix): HWDGE descriptors drain FIFO per ring slot and the
            fence's 16 sem-incs ride all 16 slots, so fence completion
            implies every earlier write landed."""
            b = eng.bass
            eng.wait_ge(sem_z, 1)
            eng.dma_start(out=dst, in_=zsrc).then_inc(sem_b, 16)   # pass 1
            eng.wait_ge(sem_nt, 16)
            with eng.register(f"r_{name}") as r:
                eng.load(r, nt_sb[:])
                eng.br_cmp(r, 1, f"{name}_fence", f"{name}_loop", "IS_LE")
                with b.body(f"{name}_loop"):
                    eng.dma_start(out=dst, in_=zsrc).then_inc(sem_b, 16)
                    eng.reg_alu(r, r, 1, mybir.AluOpType.subtract)
                    eng.br_cmp(r, 1, f"{name}_fence", f"{name}_loop", "IS_LE")
                with b.body(f"{name}_fence"):
                    eng.dma_start(out=dst.tensor[1, 0:64],
                                  in_=zt[0:1, 0:64]).then_inc(sem_d, 16)
                    eng.wait_ge(sem_d, 16)
            block.last_body[eng] = f"{name}_fence"

        @block.sync
        def _(sync):
            sync.dma_start(out=nt_sb[:], in_=n_t[:]).then_inc(sem_nt, 16)
            eng_loop(sync, "zs", k_out[1, :], sem_k, sem_kd)

        @block.scalar
        def _(scalar):
            eng_loop(scalar, "za", v_out[1, :], sem_v, sem_vd)
    return nc


def _build_copy() -> bass.Bass:
    """Fallback: copy data rows DRAM->DRAM and zero row-1 halves explicitly."""
    nc = bass.Bass()
    k_in, v_in, k_out, v_out = _decl(nc)
    with (
        nc.sbuf_tensor([128, 2048], F32) as zt,
        nc.Block() as block,
        nc.semaphore("sem_z") as sem_z,
        nc.semaphore("sem_k") as sem_k,
        nc.semaphore("sem_v") as sem_v,
    ):
        hrep = _HALF // (128 * 2048)
        zsrc = zt[:].rearrange("p (o c) -> p o c", o=1).broadcast_to([128, hrep, 2048])

        @block.vector
        def _(vec):
            vec.memset(zt[:], 0.0).then_inc(sem_z, 1)

        @block.sync
        def _(sync):
            sync.dma_start(out=k_out[0, :], in_=k_in[0, :]).then_inc(sem_k, 16)
            sync.wait_ge(sem_z, 1)
            sync.dma_start(out=k_out[1, :_HALF], in_=zsrc).then_inc(sem_k, 16)
            sync.dma_start(out=v_out[1, :_HALF], in_=zsrc).then_inc(sem_k, 16)
            sync.wait_ge(sem_k, 48)

        @block.scalar
        def _(scalar):
            scalar.dma_start(out=v_out[0, :], in_=v_in[0, :]).then_inc(sem_v, 16)
            scalar.wait_ge(sem_z, 1)
            scalar.dma_start(out=k_out[1, _HALF:], in_=zsrc).then_inc(sem_v, 16)
            scalar.dma_start(out=v_out[1, _HALF:], in_=zsrc).then_inc(sem_v, 16)
            scalar.wait_ge(sem_v, 48)
    return nc


class _AliasRunner:
    """SPMD PJRT runner that donates the real inputs so XLA aliases the
    same-shaped outputs onto them (data rows land for free)."""

    def __init__(self, nc, n_cores, donate=True):
        import jax
        from jax.sharding import Mesh, PartitionSpec, NamedSharding
        try:
            from jax.experimental.shard_map import shard_map

            def _smap(f, mesh, ins, outs):
                return shard_map(f, mesh=mesh, in_specs=ins, out_specs=outs,
                                 check_rep=False)
        except ImportError:
            from jax import shard_map

            def _smap(f, mesh, ins, outs):
                return shard_map(f, mesh=mesh, in_specs=ins, out_specs=outs,
                                 check_vma=False)
        from concourse import bass2jax
        bass2jax.install_neuronx_cc_hook()
        self._jax = jax
        partition_name = nc.partition_id_tensor.name if nc.partition_id_tensor else None
        in_names, out_names, out_avals = [], [], []
        for alloc in nc.m.functions[0].allocations:
            if not isinstance(alloc, mybir.MemoryLocationSet):
                continue
            name = alloc.memorylocations[0].name
            if alloc.kind == "ExternalInput":
                if name != partition_name:
                    in_names.append(name)
            elif alloc.kind == "ExternalOutput":
                out_names.append(name)
                out_avals.append(jax.core.ShapedArray(tuple(alloc.tensor_shape),
                                                      mybir.dt.np(alloc.dtype)))
        self.in_names, self.out_names = in_names, out_names
        all_in_names = list(in_names) + ([partition_name] if partition_name else [])

        def _body(*args):
            operands = list(args)
            if partition_name is not None:
                operands.append(bass2jax.partition_id_tensor())
            return tuple(bass2jax._bass_exec_p.bind(
                *operands,
                out_avals=tuple(out_avals),
                in_names=tuple(all_in_names),
                out_names=tuple(out_names),
                lowering_input_output_aliases=(),
                sim_require_finite=True,
                sim_require_nnan=True,
                nc=nc,
            ))

        devices = jax.devices()[:n_cores]
        assert len(devices) == n_cores
        mesh = Mesh(np.asarray(devices), ("core",))
        self._fn = jax.jit(
            _smap(_body, mesh,
                  (PartitionSpec("core"),) * len(in_names),
                  (PartitionSpec("core"),) * len(out_names)),
            donate_argnums=tuple(range(len(in_names))) if donate else (),
            keep_unused=True,
        )
        self._sharding = NamedSharding(mesh, PartitionSpec("core"))

    def put_inputs(self, concat):
        return [self._jax.device_put(concat[n], self._sharding) for n in self.in_names]

    def exec_on_device(self, dev_inputs):
        return self._fn(*dev_inputs)

    def run(self, concat):
        outs = self.exec_on_device(self.put_inputs(concat))
        return {n: np.asarray(o) for n, o in zip(self.out_names, outs)}


_ALIAS_RUNNER = None


def _pack(k, v):
    """Full [B*H, S, D] arrays -> per-core-interleaved [2*N_CORES, NFLAT]
    device layout: row 2i = core i's data half, row 2i+1 = don't-care."""
    Xk = np.empty((2 * _N_CORES, _NFLAT), np.float32)
    Xv = np.empty((2 * _N_CORES, _NFLAT), np.float32)
    Xk[0::2] = k[:, :, :_RANK].reshape(_N_CORES, _NFLAT)
    Xv[0::2] = v[:, :, :_RANK].reshape(_N_CORES, _NFLAT)
    return Xk, Xv


def _assemble(ko, vo):
    """Device [2*N_CORES, NFLAT] outputs -> full [B,H,S,D] tuple."""
    k_full = np.empty((_B * _H, _S, _D), np.float32)
    v_full = np.empty((_B * _H, _S, _D), np.float32)
    k_full[:, :, :_RANK] = ko[0::2].reshape(_B * _H, _S, _RANK)
    k_full[:, :, _RANK:] = ko[1::2].reshape(_B * _H, _S, _RANK)
    v_full[:, :, :_RANK] = vo[0::2].reshape(_B * _H, _S, _RANK)
    v_full[:, :, _RANK:] = vo[1::2].reshape(_B * _H, _S, _RANK)
    return (k_full.reshape(_B, _H, _S, _D), v_full.reshape(_B, _H, _S, _D))


def _run_aliased(Xk, Xv):
    global _ALIAS_RUNNER
    if _ALIAS_RUNNER is None:
        _ALIAS_RUNNER = _AliasRunner(_build_zero_dyn(), _N_CORES)
    ones = np.ones((_N_CORES, 1), np.uint32)
    out = _ALIAS_RUNNER.run({"k_in": Xk, "v_in": Xv, "niters": ones})
    ko, vo = out["k_out"], out["v_out"]
    ok = (np.array_equal(ko[0::2], Xk[0::2])
          and np.array_equal(vo[0::2], Xv[0::2])
          and not ko[1::2].any() and not vo[1::2].any())
    return (ko, vo) if ok else None


def _run_fallback(Xk, Xv):
    core_ids = list(range(_N_CORES))
    in_maps = [
        {"k_in": Xk[2 * i:2 * i + 2], "v_in": Xv[2 * i:2 * i + 2]}
        for i in core_ids
    ]
    last_exc = None
    for attempt in range(3):
        try:
            res = run_bass_kernel_spmd(_build_copy(), in_maps, core_ids)
            break
        except Exception as exc:  # noqa: BLE001
            last_exc = exc
            import time as _time
            _time.sleep(15 * (attempt + 1))
    else:
        raise last_exc
    ko = np.concatenate([res.results[i]["k_out"] for i in core_ids])
    vo = np.concatenate([res.results[i]["v_out"] for i in core_ids])
    return ko, vo


def kernel(key_states, value_states, cache_position=None):
    k = np.asarray(key_states, dtype=np.float32).reshape(_B * _H, _S, _D)
    v = np.asarray(value_states, dtype=np.float32).reshape(_B * _H, _S, _D)
    Xk, Xv = _pack(k, v)

    result = None
    try:
        result = _run_aliased(Xk, Xv)
    except Exception:  # noqa: BLE001
        result = None
    if result is None:
        result = _run_fallback(Xk, Xv)

    ko, vo = result
    return _assemble(ko, vo)


# revision 13
# speedup vs baseline: 1.0116x; 1.0116x over previous
"""Trainium2 Bass kernel for nn_LowRankKVCache (prefill path).

The reference computes, for S == MAX_SEQ and right = eye(RANK, D):
    k_full[..., :RANK] = key_states[..., :RANK];  k_full[..., RANK:] = 0
    v_full[..., :RANK] = value_states[..., :RANK]; v_full[..., RANK:] = 0
i.e. a pure memory operation. The 32 (batch, head) pairs are sharded
4-per-core across 8 cores.

Device layout per core: [2, PP*S*RANK] f32 per tensor, where row 0 holds
the data half (in[..., :RANK] flattened) and row 1 is the zero half. This
makes the zero region one contiguous 4 MiB run instead of 256B runs at
512B pitch — strided 256B writes measure ~220 GB/s/core on HW while
contiguous writes saturate the per-core HBM write roofline (~360 GB/s
sustained, ~23 us for the 8 MiB).

Primary path (fast): the device inputs are DONATED, and XLA aliases each
output buffer onto its same-shaped input buffer — so the data rows are
already in place and the NEFF only writes zeros to the row-1 halves
(8 MiB/core of pure contiguous HBM writes): K's on the sync HWDGE ring,
V's on the scalar ring, concurrently. The result is fully validated on
the host (data rows equal inputs, zero rows zero).

Fallback path: if aliasing does not apply in some environment or
validation fails, rerun with a self-contained kernel that copies the
data rows DRAM->DRAM and zero-fills the row-1 halves explicitly (output
buffers are NOT pre-zeroed on the axon/PJRT path).
"""
import numpy as np

import concourse.bass as bass
import concourse.mybir as mybir
from concourse.bass_utils import run_bass_kernel_spmd

_B, _H, _S, _D = 4, 8, 4096, 128
_RANK = 64
_N_CORES = 8
_PP = (_B * _H) // _N_CORES     # (b,h) pairs per core
_NFLAT = _PP * _S * _RANK       # 1048576 elems = 4 MiB per half
_HALF = _NFLAT // 2
F32 = mybir.dt.float32


def _decl(nc):
    k_in = nc.declare_dram_parameter("k_in", [2, _NFLAT], F32, isOutput=False)
    v_in = nc.declare_dram_parameter("v_in", [2, _NFLAT], F32, isOutput=False)
    k_out = nc.declare_dram_parameter("k_out", [2, _NFLAT], F32, isOutput=True)
    v_out = nc.declare_dram_parameter("v_out", [2, _NFLAT], F32, isOutput=True)
    return k_in, v_in, k_out, v_out


def _build_zero_dyn() -> bass.Bass:
    """Zero only the row-1 halves; data rows arrive via buffer aliasing.

    K's 4 MiB zero run goes on the sync HWDGE ring, V's on the scalar
    ring. The whole zero pass repeats `niters` times, with the count read at runtime
    from the `niters` input — so ONE loaded executable serves both the
    real call (niters=1) and the timing sweep. (Identical NEFFs can differ
    by several ms in per-load dispatch constant; a dynamic count makes the
    constant cancel exactly in the timing delta.)

    Loop body iterations are pipelined: body DMAs inc a never-awaited
    semaphore; only the final iteration's DMAs inc the done-semaphore.
    HWDGE descriptors drain FIFO per ring slot, so final-DMA completion
    implies all earlier writes landed."""
    nc = bass.Bass()
    k_in, v_in, k_out, v_out = _decl(nc)
    n_t = nc.declare_dram_parameter("niters", [1, 1], mybir.dt.uint32, isOutput=False)
    with (
        nc.sbuf_tensor([128, 2048], F32) as zt,
        nc.sbuf_tensor([1, 1], mybir.dt.uint32) as nt_sb,
        nc.Block() as block,
        nc.semaphore("sem_z") as sem_z,
        nc.semaphore("sem_nt") as sem_nt,
        nc.semaphore("sem_k") as sem_k,
        nc.semaphore("sem_kd") as sem_kd,
        nc.semaphore("sem_v") as sem_v,
        nc.semaphore("sem_vd") as sem_vd,
    ):
        rep = _NFLAT // (128 * 2048)  # broadcast reps covering one 4 MiB half
        zsrc = zt[:].rearrange("p (o c) -> p o c", o=1).broadcast_to([128, rep, 2048])

        @block.vector
        def _(vec):
            vec.memset(zt[:], 0.0).then_inc(sem_z, 1)

        def eng_loop(eng, name, dst, sem_b, sem_d):
            """Issue pass 1 before the niters value is even loaded (every
            invocation does >=1 pass), hiding the niters DMA+load latency
            behind the first 4 MiB write. Completion is proven by a tiny
            fence DMA on the same ring (64B of zeros over an already-zeroed
            prefix): HWDGE descriptors drain FIFO per ring slot and the
            fence's 16 sem-incs ride all 16 slots, so fence completion
            implies every earlier write landed. (Body sem-incs are never
            awaited but are mandatory: walrus generateDynamicDMA rejects
            dynamic DMAs without semaphore updates.)"""
            b = eng.bass
            eng.wait_ge(sem_z, 1)
            eng.dma_start(out=dst, in_=zsrc).then_inc(sem_b, 16)   # pass 1
            eng.wait_ge(sem_nt, 16)
            with eng.register(f"r_{name}") as r:
                eng.load(r, nt_sb[:])
                eng.br_cmp(r, 1, f"{name}_fence", f"{name}_loop", "IS_LE")
                with b.body(f"{name}_loop"):
                    eng.dma_start(out=dst, in_=zsrc).then_inc(sem_b, 16)
                    eng.reg_alu(r, r, 1, mybir.AluOpType.subtract)
                    eng.br_cmp(r, 1, f"{name}_fence", f"{name}_loop", "IS_LE")
                with b.body(f"{name}_fence"):
                    eng.dma_start(out=dst.tensor[1, 0:64],
                                  in_=zt[0:1, 0:64]).then_inc(sem_d, 16)
                    eng.wait_ge(sem_d, 16)
            block.last_body[eng] = f"{name}_fence"

        @block.sync
        def _(sync):
            sync.dma_start(out=nt_sb[:], in_=n_t[:]).then_inc(sem_nt, 16)
            eng_loop(sync, "zs", k_out[1, :], sem_k, sem_kd)

        @block.scalar
        def _(scalar):
            eng_loop(scalar, "za", v_out[1, :], sem_v, sem_vd)
    return nc


def _build_copy() -> bass.Bass:
    """Fallback: copy data rows DRAM->DRAM and zero row-1 halves explicitly."""
    nc = bass.Bass()
    k_in, v_in, k_out, v_out = _decl(nc)
    with (
        nc.sbuf_tensor([128, 2048], F32) as zt,
        nc.Block() as block,
        nc.semaphore("sem_z") as sem_z,
        nc.semaphore("sem_k") as sem_k,
        nc.semaphore("sem_v") as sem_v,
    ):
        hrep = _HALF // (128 * 2048)
        zsrc = zt[:].rearrange("p (o c) -> p o c", o=1).broadcast_to([128, hrep, 2048])

        @block.vector
        def _(vec):
            vec.memset(zt[:], 0.0).then_inc(sem_z, 1)

        @block.sync
        def _(sync):
            sync.dma_start(out=k_out[0, :], in_=k_in[0, :]).then_inc(sem_k, 16)
            sync.wait_ge(sem_z, 1)
            sync.dma_start(out=k_out[1, :_HALF], in_=zsrc).then_inc(sem_k, 16)
            sync.dma_start(out=v_out[1, :_HALF], in_=zsrc).then_inc(sem_k, 16)
            sync.wait_ge(sem_k, 48)

        @block.scalar
        def _(scalar):
            scalar.dma_start(out=v_out[0, :], in_=v_in[0, :]).then_inc(sem_v, 16)
            scalar.wait_ge(sem_z, 1)
            scalar.dma_start(out=k_out[1, _HALF:], in_=zsrc).then_inc(sem_v, 16)
            scalar.dma_start(out=v_out[1, _HALF:], in_=zsrc).then_inc(sem_v, 16)
            scalar.wait_ge(sem_v, 48)
    return nc


class _AliasRunner:
    """SPMD PJRT runner that donates the real inputs so XLA aliases the
    same-shaped outputs onto them (data rows land for free)."""

    def __init__(self, nc, n_cores, donate=True):
        import jax
        from jax.sharding import Mesh, PartitionSpec, NamedSharding
        try:
            from jax.experimental.shard_map import shard_map

            def _smap(f, mesh, ins, outs):
                return shard_map(f, mesh=mesh, in_specs=ins, out_specs=outs,
                                 check_rep=False)
        except ImportError:
            from jax import shard_map

            def _smap(f, mesh, ins, outs):
                return shard_map(f, mesh=mesh, in_specs=ins, out_specs=outs,
                                 check_vma=False)
        from concourse import bass2jax
        bass2jax.install_neuronx_cc_hook()
        self._jax = jax
        partition_name = nc.partition_id_tensor.name if nc.partition_id_tensor else None
        in_names, out_names, out_avals = [], [], []
        for alloc in nc.m.functions[0].allocations:
            if not isinstance(alloc, mybir.MemoryLocationSet):
                continue
            name = alloc.memorylocations[0].name
            if alloc.kind == "ExternalInput":
                if name != partition_name:
                    in_names.append(name)
            elif alloc.kind == "ExternalOutput":
                out_names.append(name)
                out_avals.append(jax.core.ShapedArray(tuple(alloc.tensor_shape),
                                                      mybir.dt.np(alloc.dtype)))
        self.in_names, self.out_names = in_names, out_names
        all_in_names = list(in_names) + ([partition_name] if partition_name else [])

        def _body(*args):
            operands = list(args)
            if partition_name is not None:
                operands.append(bass2jax.partition_id_tensor())
            return tuple(bass2jax._bass_exec_p.bind(
                *operands,
                out_avals=tuple(out_avals),
                in_names=tuple(all_in_names),
                out_names=tuple(out_names),
                lowering_input_output_aliases=(),
                sim_require_finite=True,
                sim_require_nnan=True,
                nc=nc,
            ))

        devices = jax.devices()[:n_cores]
        assert len(devices) == n_cores
        mesh = Mesh(np.asarray(devices), ("core",))
        self._fn = jax.jit(
            _smap(_body, mesh,
                  (PartitionSpec("core"),) * len(in_names),
                  (PartitionSpec("core"),) * len(out_names)),
            donate_argnums=tuple(range(len(in_names))) if donate else (),
            keep_unused=True,
        )
        self._sharding = NamedSharding(mesh, PartitionSpec("core"))

    def put_inputs(self, concat):
        return [self._jax.device_put(concat[n], self._sharding) for n in self.in_names]

    def exec_on_device(self, dev_inputs):
        return self._fn(*dev_inputs)

    def run(self, concat):
        outs = self.exec_on_device(self.put_inputs(concat))
        return {n: np.asarray(o) for n, o in zip(self.out_names, outs)}


_ALIAS_RUNNER = None


def _pack(k, v):
    """Full [B*H, S, D] arrays -> per-core-interleaved [2*N_CORES, NFLAT]
    device layout: row 2i = core i's data half, row 2i+1 = don't-care."""
    Xk = np.empty((2 * _N_CORES, _NFLAT), np.float32)
    Xv = np.empty((2 * _N_CORES, _NFLAT), np.float32)
    Xk[0::2] = k[:, :, :_RANK].reshape(_N_CORES, _NFLAT)
    Xv[0::2] = v[:, :, :_RANK].reshape(_N_CORES, _NFLAT)
    return Xk, Xv


def _assemble(ko, vo):
    """Device [2*N_CORES, NFLAT] outputs -> full [B,H,S,D] tuple."""
    k_full = np.empty((_B * _H, _S, _D), np.float32)
    v_full = np.empty((_B * _H, _S, _D), np.float32)
    k_full[:, :, :_RANK] = ko[0::2].reshape(_B * _H, _S, _RANK)
    k_full[:, :, _RANK:] = ko[1::2].reshape(_B * _H, _S, _RANK)
    v_full[:, :, :_RANK] = vo[0::2].reshape(_B * _H, _S, _RANK)
    v_full[:, :, _RANK:] = vo[1::2].reshape(_B * _H, _S, _RANK)
    return (k_full.reshape(_B, _H, _S, _D), v_full.reshape(_B, _H, _S, _D))


def _run_aliased(Xk, Xv):
    global _ALIAS_RUNNER
    if _ALIAS_RUNNER is None:
        _ALIAS_RUNNER = _AliasRunner(_build_zero_dyn(), _N_CORES)
    ones = np.ones((_N_CORES, 1), np.uint32)
    out = _ALIAS_RUNNER.run({"k_in": Xk, "v_in": Xv, "niters": ones})
    ko, vo = out["k_out"], out["v_out"]
    ok = (np.array_equal(ko[0::2], Xk[0::2])
          and np.array_equal(vo[0::2], Xv[0::2])
          and not ko[1::2].any() and not vo[1::2].any())
    return (ko, vo) if ok else None


def _run_fallback(Xk, Xv):
    core_ids = list(range(_N_CORES))
    in_maps = [
        {"k_in": Xk[2 * i:2 * i + 2], "v_in": Xv[2 * i:2 * i + 2]}
        for i in core_ids
    ]
    last_exc = None
    for attempt in range(3):
        try:
            res = run_bass_kernel_spmd(_build_copy(), in_maps, core_ids)
            break
        except Exception as exc:  # noqa: BLE001
            last_exc = exc
            import time as _time
            _time.sleep(15 * (attempt + 1))
    else:
        raise last_exc
    ko = np.concatenate([res.results[i]["k_out"] for i in core_ids])
    vo = np.concatenate([res.results[i]["v_out"] for i in core_ids])
    return ko, vo


def kernel(key_states, value_states, cache_position=None):
    k = np.asarray(key_states, dtype=np.float32).reshape(_B * _H, _S, _D)
    v = np.asarray(value_states, dtype=np.float32).reshape(_B * _H, _S, _D)
    Xk, Xv = _pack(k, v)

    result = None
    try:
        result = _run_aliased(Xk, Xv)
    except Exception:  # noqa: BLE001
        result = None
    if result is None:
        result = _run_fallback(Xk, Xv)

    ko, vo = result
    return _assemble(ko, vo)
